# revision 39
# baseline (speedup 1.0000x reference)
"""Two-layer GAT (PyG-style, eval mode) on 8 Trainium2 NeuronCores.

Strategy (dst-sharded, per-node edge columns):
  - Host: shard destination nodes into 8 contiguous ranges (one per core).
    Within a core, nodes are permuted by in-degree so that each window of 128
    nodes has near-uniform degree; window w stores its edges as K[w] columns
    of 128 rows, where row p holds the in-edges of the window's node p
    (padded to K[w] = max in-degree in the window).  Self-loops are excluded
    from the edge lists (their features are local).
  - Rotated feature basis: per head, an orthogonal Householder Q_h whose
    first row is a1_src[h]/||a1_src[h]|| is folded into W1 (y = h1 @ Q^T), so
    alpha_src of a record is just feature column h*C times ||a1_src[h]||.
    Layer-1 records are then exactly 512 B (256 bf16 features, nothing else);
    alpha_dst values (pre-divided by the norms) stay core-local in SBUF.
    The inverse rotation (y @ Q) is a tiny per-window PE matmul applied
    before the ELU.
  - Device phase A: per-core rows of y = x @ (W1 @ Q^T) via PE matmuls,
    written to DRAM (and kept in SBUF for the self-loop terms), then ONE
    Shared-output AllGather (~200+ GB/s) of the 512B-row table.
  - Device phase B (layer-1 edge phase, per window): one dma_gather of
    records by edge source (~8 ns/row Pool-engine hold = the bottleneck);
    ex = exp(norm_h * leaky_relu(y0 + ad')) with no segment-max shift (edge
    logits are bounded here).  Pad edge columns gather a crafted pad row
    whose y0 column is -200/norm, contributing exp(-40) ~ 0.  The weighted
    feature sum reduces over edge columns with an in-place bf16 halving
    tree (contiguous adds), then unrotate, ELU, and produce layer-2 records
    rec2 = [h @ W2 | alpha2_src | alpha2_dst] (fp32, 256B rows).
  - tab2's AllGather is split into geometrically staggered chunks (Local
    address space) fired as phase-B windows complete, each dispatched a few
    windows late so the Pool queue never head-of-line blocks; phase D
    repeats the edge phase for layer 2, and log_softmax runs as one batched
    tail pass.  The host inverse-permutes the rows.

The dominant device cost is SWDGE descriptor emission for the by-source
gathers (~8 ns/edge of Pool-engine hold, invariant to element size), which
this layout minimizes: one gathered row per non-self edge per layer.
"""

import sys

for _p in ("/opt/trn_rl_repo", "/root/.axon_site/_ro/trn_rl_repo"):
    if _p not in sys.path:
        sys.path.append(_p)

import numpy as np
from ml_dtypes import bfloat16 as _BF16NP

import concourse.bass as bass
import concourse.mybir as mybir
import concourse.tile as tile
from concourse import bacc
from concourse.bass_utils import run_bass_kernel_spmd
from concourse.masks import make_identity

F32 = mybir.dt.float32
BF16 = mybir.dt.bfloat16
I16 = mybir.dt.int16
ALU = mybir.AluOpType
ACTF = mybir.ActivationFunctionType
AXX = mybir.AxisListType.X

CORES = 8
PW = 128           # nodes per window
NEG_SLOPE = 0.2

_CACHE = {}


# --------------------------------------------------------------------------
# host-side preprocessing
# --------------------------------------------------------------------------

def _wrap_idx(vals):
    """Wrap a flat index vector into the [128, n/16] layout dma_gather wants
    (index i at partition i%16, col i//16; replicated to all 8 Q7 groups)."""
    w = vals.reshape(-1, 16).T.astype(np.int16)
    return np.ascontiguousarray(np.tile(w, (8, 1)))


def _prep(x, edge_index, W1, a1_src, a1_dst, b1, W2, a2_src, a2_dst, b2):
    N, F = x.shape
    H, C = a1_src.shape
    OUT = W2.shape[1]
    NPC = N // CORES
    W = (NPC + PW - 1) // PW
    NPCP = W * PW
    if NPCP == NPC:
        NPCP += PW          # spare pad block (holds the pad row)

    src0 = np.asarray(edge_index[0], np.int64)
    dst0 = np.asarray(edge_index[1], np.int64)

    deg = np.zeros(N, np.int64)
    np.add.at(deg, dst0, 1)

    # per-core degree-sorted node permutation; perm[new_global_row] = node id
    perm = np.empty(N, np.int64)
    for c in range(CORES):
        ids = np.arange(c * NPC, (c + 1) * NPC)
        perm[c * NPC:(c + 1) * NPC] = ids[np.argsort(deg[ids], kind="stable")]
    newrow = np.empty(N, np.int64)
    newrow[perm] = np.arange(N)          # node id -> permuted global row
    # tab1: single Shared-output AllGather.  tab2: chunked Local AllGathers,
    # rows laid out [chunk][core][row-in-chunk]; chunks complete in reversed
    # window order (phase B), geometrically staggered so the last-fired
    # chunks are small.
    RB1 = [0, NPCP]
    RB2 = sorted(set(min(b * PW, NPCP) for b in (0, 2, 4, 8, 16)) | {NPCP})

    def _mkrow(bounds):
        b = np.asarray(bounds, np.int64)
        base = np.concatenate([[0], np.cumsum((b[1:] - b[:-1]) * CORES)])[:-1]
        _c = newrow // NPC
        _r = newrow % NPC
        j = np.searchsorted(b, _r, side="right") - 1
        return base[j] + _c * (b[j + 1] - b[j]) + (_r - b[j]), b, base

    tabrow, B1a, B1base = _mkrow(RB1)
    tabrow2, B2a, B2base = _mkrow(RB2)

    def _rowof(bounds, base, r):
        j = int(np.searchsorted(bounds, r, side="right") - 1)
        return base[j], int(bounds[j + 1] - bounds[j]), j, int(bounds[j])

    # per-window max degree (shared across cores for SPMD uniformity)
    degw = deg[perm].reshape(CORES, NPC)
    K = []
    for w in range(W):
        lo, hi = w * PW, min((w + 1) * PW, NPC)
        K.append(max(1, int(degw[:, lo:hi].max())))
    K = np.asarray(K, np.int64)
    CK = np.concatenate([[0], np.cumsum(K)])   # column offsets
    SK = int(K.sum())
    EPC = SK * PW                              # gather indices per core/layer

    # in-edges grouped by (permuted) destination row: CSR over new rows
    order = np.argsort(newrow[dst0], kind="stable")
    src_s = src0[order]
    starts = np.zeros(N + 1, np.int64)
    np.add.at(starts, newrow[dst0] + 1, 1)
    starts = np.cumsum(starts)

    pb1, ps1, _, pl1 = _rowof(B1a, B1base, NPC)
    pb2, ps2, _, pl2 = _rowof(B2a, B2base, NPC)
    padtok1 = pb1 + np.arange(CORES) * ps1 + (NPC - pl1)
    padtok2 = pb2 + np.arange(CORES) * ps2 + (NPC - pl2)

    # Layer-2 region split: for the largest (first-processed) phase-D
    # windows, edges whose source row is >= RSPLIT (available before the
    # last tab2 chunks land) go in a leading column block gathered early.
    RSPLIT = RB2[-3] if len(RB2) >= 3 else 0
    TSPLIT = CORES * RSPLIT
    SPLITW = [W - 1, W - 2] if (W >= 2 and RSPLIT and NPC > RSPLIT) else []
    assert all(padtok2 >= TSPLIT)

    isrc = np.zeros((CORES, EPC), np.int64)
    for c in range(CORES):
        isrc[c, :] = padtok1[c]
        for w in range(W):
            for p in range(PW):
                lp = w * PW + p
                if lp >= NPC:
                    continue
                r = c * NPC + lp
                s, e = starts[r], starts[r + 1]
                cols = CK[w] + np.arange(e - s)
                isrc[c, cols * PW + p] = tabrow[src_s[s:e]]
    isrc_w = np.stack([_wrap_idx(isrc[c]) for c in range(CORES)])

    # per-window L2 column counts (split windows get KA | KB blocks)
    K2 = [int(k) for k in K]
    KA = {}
    for w in SPLITW:
        ka = kb = 1
        for c in range(CORES):
            for p in range(PW):
                lp = w * PW + p
                if lp >= NPC:
                    continue
                r = c * NPC + lp
                s, e = starts[r], starts[r + 1]
                na = int(np.sum(tabrow2[src_s[s:e]] >= TSPLIT))
                ka = max(ka, na)
                kb = max(kb, (e - s) - na)
        KA[w] = ka
        K2[w] = ka + kb
    CK2 = np.concatenate([[0], np.cumsum(K2)])
    EPC2 = int(CK2[-1]) * PW

    isrc2 = np.zeros((CORES, EPC2), np.int64)
    for c in range(CORES):
        for w in range(W):
            base = CK2[w] * PW
            kw2 = K2[w]
            if w in SPLITW:
                # A block cols [0,KA): idx relative to TSPLIT; pads -> pad tok
                isrc2[c, base:base + kw2 * PW] = padtok2[c]
                isrc2[c, base:base + KA[w] * PW] = padtok2[c] - TSPLIT
            else:
                isrc2[c, base:base + kw2 * PW] = padtok2[c]
            for p in range(PW):
                lp = w * PW + p
                if lp >= NPC:
                    continue
                r = c * NPC + lp
                s, e = starts[r], starts[r + 1]
                tr = tabrow2[src_s[s:e]]
                if w in SPLITW:
                    ta = tr[tr >= TSPLIT] - TSPLIT
                    tb = tr[tr < TSPLIT]
                    ca = np.arange(len(ta))
                    cb = KA[w] + np.arange(len(tb))
                    isrc2[c, base + ca * PW + p] = ta
                    isrc2[c, base + cb * PW + p] = tb
                else:
                    cols = np.arange(e - s)
                    isrc2[c, base + cols * PW + p] = tr
    isrc2_w = np.stack([_wrap_idx(isrc2[c]) for c in range(CORES)])

    # per-head Householder rotation: Q_h rows orthonormal, row 0 is
    # a1_src[h]/||a1_src[h]||.  y = h1 @ Q^T, h1 = y @ Q.
    a1s64 = np.asarray(a1_src, np.float64)
    norms = np.linalg.norm(a1s64, axis=1)            # [H]
    Qblk = np.zeros((F, F), np.float64)
    for h in range(H):
        q0 = a1s64[h] / norms[h]
        v = -q0.copy()
        v[0] += 1.0
        nv = np.linalg.norm(v)
        if nv < 1e-12:
            Qh = np.eye(C)
        else:
            v /= nv
            Qh = np.eye(C) - 2.0 * np.outer(v, v)
        Qblk[h * C:(h + 1) * C, h * C:(h + 1) * C] = Qh

    # folded weight matrices (f64 for exactness of the tiny folds)
    Ad = np.zeros((F, H), np.float64)
    for h in range(H):
        Ad[h * C:(h + 1) * C, h] = np.asarray(a1_dst, np.float64)[h]
    W1_64 = np.asarray(W1, np.float64)
    wc1 = np.concatenate(
        [W1_64 @ Qblk.T,
         (W1_64 @ Ad) / norms[None, :]], 1).astype(np.float32)  # [F, F+H]
    AF = F + H
    W2_64 = np.asarray(W2, np.float64)
    wc2 = np.concatenate(
        [np.asarray(W2, np.float32),
         (W2_64 @ np.asarray(a2_src[0], np.float64))[:, None].astype(np.float32),
         (W2_64 @ np.asarray(a2_dst[0], np.float64))[:, None].astype(np.float32)],
        1)                                               # [F, OUT+2]

    padrow = np.zeros((PW, F), np.float32)
    for h in range(H):
        padrow[:, h * C] = -200.0 / norms[h]

    KT = F // 128
    xt = np.zeros((CORES, KT, 128, NPCP), np.float32)
    xp = np.asarray(x, np.float32)[perm]
    for c in range(CORES):
        xt[c, :, :, :NPC] = (xp[c * NPC:(c + 1) * NPC].T
                             .reshape(KT, 128, NPC))

    b1q = None
    if np.any(np.asarray(b1)):
        b1q = (np.asarray(b1, np.float64) @ Qblk.T).astype(np.float32)

    host = {
        "N": N, "F": F, "H": H, "C": C, "OUT": OUT,
        "NPC": NPC, "W": W, "NPCP": NPCP,
        "K": tuple(int(k) for k in K), "SK": SK, "EPC": EPC,
        "RB1": tuple(int(b) for b in RB1), "RB2": tuple(int(b) for b in RB2),
        "K2": tuple(int(k) for k in K2), "EPC2": EPC2,
        "KA": {int(w): int(KA[w]) for w in KA}, "RSPLIT": int(RSPLIT),
        "perm": perm,
        "use_b1": b1q is not None, "use_b2": bool(np.any(b2)),
    }
    in_maps = []
    for c in range(CORES):
        m = {
            "xt": xt[c].astype(_BF16NP),
            "wc1": np.ascontiguousarray(
                wc1.reshape(KT, F // KT, AF)).astype(_BF16NP),
            "wc2": np.ascontiguousarray(wc2.reshape(KT, F // KT, OUT + 2)),
            "qblk": np.ascontiguousarray(
                Qblk.reshape(KT, F // KT, F).astype(np.float32)),
            "normsrep": np.broadcast_to(
                norms.astype(np.float32), (PW, H)).copy(),
            "padrow": padrow.astype(_BF16NP),
            "isrc": isrc_w[c],
            "isrc2": isrc2_w[c],
        }
        if b1q is not None:
            m["b1rep"] = np.broadcast_to(b1q, (PW, F)).copy()
        if host["use_b2"]:
            m["b2rep"] = np.broadcast_to(np.asarray(b2, np.float32), (PW, OUT)).copy()
        in_maps.append(m)
    return host, in_maps


# --------------------------------------------------------------------------
# device kernel
# --------------------------------------------------------------------------

def _build(hp):
    F, H, C, OUT = hp["F"], hp["H"], hp["C"], hp["OUT"]
    W, NPCP = hp["W"], hp["NPCP"]
    K, SK, EPC = hp["K"], hp["SK"], hp["EPC"]
    K2, EPC2 = hp["K2"], hp["EPC2"]
    KA, RSPLIT = hp["KA"], hp["RSPLIT"]
    TSPLIT = CORES * RSPLIT
    KMAX = max(max(K), max(K2))
    KT = F // 128               # contraction tiles (2)
    REC1 = F                    # 256 bf16 units = 512 B rows
    AF = F + H                  # 264 matmul output cols (y | ad')
    REC2 = 64                   # 256B rows: OUT+2 valid f32 cols of rec2
    A2 = OUT + 2
    NTAB = CORES * NPCP
    EPC16 = EPC // 16
    RB1 = hp["RB1"]
    RB2 = hp["RB2"]
    NPC = hp["NPC"]
    CK = [0]
    for k in K:
        CK.append(CK[-1] + k)
    CK2 = [0]
    for k in K2:
        CK2.append(CK2[-1] + k)

    nc = bacc.Bacc(None, target_bir_lowering=False)

    xt_p = nc.declare_dram_parameter("xt", [KT, 128, NPCP], BF16, isOutput=False)
    wc1_p = nc.declare_dram_parameter("wc1", [KT, 128, AF], BF16, isOutput=False)
    wc2_p = nc.declare_dram_parameter("wc2", [KT, 128, A2], F32, isOutput=False)
    qblk_p = nc.declare_dram_parameter("qblk", [KT, 128, F], F32, isOutput=False)
    norms_p = nc.declare_dram_parameter("normsrep", [PW, H], F32, isOutput=False)
    padrow_p = nc.declare_dram_parameter("padrow", [PW, F], BF16, isOutput=False)
    isrc_p = nc.declare_dram_parameter("isrc", [128, EPC16], I16, isOutput=False)
    isrc2_p = nc.declare_dram_parameter("isrc2", [128, EPC2 // 16], I16,
                                    isOutput=False)
    b1_p = (nc.declare_dram_parameter("b1rep", [PW, F], F32, isOutput=False)
            if hp["use_b1"] else None)
    b2_p = (nc.declare_dram_parameter("b2rep", [PW, OUT], F32, isOutput=False)
            if hp["use_b2"] else None)
    out_p = nc.declare_dram_parameter("out", [NPCP, OUT], F32, isOutput=True)

    with tile.TileContext(nc) as tc:
        with (
            tc.tile_pool(name="dram", bufs=1, space="DRAM") as dram,
            tc.tile_pool(name="const", bufs=1) as cpool,
            tc.tile_pool(name="gath", bufs=5) as gp,
            tc.tile_pool(name="gath2", bufs=3) as gp2,
            tc.tile_pool(name="mid", bufs=2) as mp,
            tc.tile_pool(name="psA", bufs=2, space="PSUM") as psA,
            tc.tile_pool(name="psB", bufs=2, space="PSUM") as psB,
            tc.tile_pool(name="psC", bufs=2, space="PSUM") as psC,
        ):
            r1loc = dram.tile([NPCP, REC1], BF16)
            tab1 = dram.tile([NTAB, REC1], BF16, addr_space="Shared")
            r2loc = dram.tile([NPCP, REC2], F32)
            tab2 = dram.tile([NTAB, REC2], F32, addr_space="Local")

            # resident constants
            wc1_sb = cpool.tile([128, KT, AF], BF16)
            for g in range(KT):
                nc.sync.dma_start(out=wc1_sb[:, g, :], in_=wc1_p[g])
            wc2_sb = cpool.tile([128, KT, A2], F32)
            for g in range(KT):
                nc.sync.dma_start(out=wc2_sb[:, g, :], in_=wc2_p[g])
            qblk_sb = cpool.tile([128, KT, F], F32)
            for g in range(KT):
                nc.sync.dma_start(out=qblk_sb[:, g, :], in_=qblk_p[g])
            norms_sb = cpool.tile([PW, H], F32)
            nc.sync.dma_start(out=norms_sb[:], in_=norms_p[:])
            padrow_sb = cpool.tile([PW, F], BF16)
            nc.sync.dma_start(out=padrow_sb[:], in_=padrow_p[:])
            ident = cpool.tile([PW, PW], F32)
            make_identity(nc, ident[:])
            xt_sb = cpool.tile([128, KT, NPCP], BF16)
            for g in range(KT):
                nc.sync.dma_start(out=xt_sb[:, g, :], in_=xt_p[g])
            isrc_sb = cpool.tile([128, EPC16], I16)
            nc.sync.dma_start(out=isrc_sb[:], in_=isrc_p[:])
            isrc2_sb = cpool.tile([128, EPC2 // 16], I16)
            nc.sync.dma_start(out=isrc2_sb[:], in_=isrc2_p[:])
            if b1_p is not None:
                b1_sb = cpool.tile([PW, F], F32)
                nc.sync.dma_start(out=b1_sb[:], in_=b1_p[:])
            if b2_p is not None:
                b2_sb = cpool.tile([PW, OUT], F32)
                nc.sync.dma_start(out=b2_sb[:], in_=b2_p[:])
            pcB = cpool.tile([PW, 2], F32)       # pad-row alphas for rec2
            nc.vector.memset(pcB[:], 0.0)
            nc.vector.memset(pcB[:, :1], -200.0)
            pzB = cpool.tile([PW, OUT], F32)
            nc.vector.memset(pzB[:], 0.0)
            ysall = cpool.tile([128, W, F], BF16)   # this core's y rows
            adall = cpool.tile([128, W, H], F32)    # this core's ad'/norm
            lgall = cpool.tile([128, W, OUT], F32)
            ezall = cpool.tile([128, W, OUT], F32)
            ssall = cpool.tile([128, W], F32)
            lsall = cpool.tile([128, W], F32)

            # ---------------- phase A: y rows for this core ---------------
            # pad rows first (never touched by the window stores)
            for lo in range(NPC, NPCP, PW):
                nr = min(PW, NPCP - lo)
                nc.sync.dma_start(out=r1loc[lo:lo + nr, :], in_=padrow_sb[:nr])
            for nt in range(W):
                NR = min(PW, NPC - nt * PW)   # don't clobber the pad rows
                rp = psB.tile([128, AF], F32, tag="acc")
                for g in range(KT):
                    nc.tensor.matmul(rp[:], lhsT=xt_sb[:, g, nt * PW:(nt + 1) * PW],
                                     rhs=wc1_sb[:, g, :],
                                     start=(g == 0), stop=(g == KT - 1))
                rsb = mp.tile([128, F], BF16, tag="rsb")      # y, bf16
                if b1_p is not None:
                    nc.vector.tensor_add(out=rsb[:], in0=rp[:, :F], in1=b1_sb[:])
                else:
                    nc.vector.tensor_copy(out=rsb[:], in_=rp[:, :F])
                nc.vector.tensor_copy(out=adall[:, nt, :], in_=rp[:, F:])
                nc.vector.tensor_copy(out=ysall[:, nt, :], in_=rsb[:])
                nc.sync.dma_start(out=r1loc[nt * PW:nt * PW + NR, :],
                                  in_=rsb[:NR])
            nc.gpsimd.collective_compute(
                "AllGather", ALU.bypass, replica_groups=[list(range(CORES))],
                ins=[r1loc[:, :].opt()], outs=[tab1[:, :].opt()])

            # r2loc pad rows up front so the first-fired tab2 chunk has them
            for lo in range(NPC, NPCP, PW):
                nr = min(PW, NPCP - lo)
                nc.sync.dma_start(out=r2loc[lo:lo + nr, :OUT], in_=pzB[:nr])
                nc.sync.dma_start(out=r2loc[lo:lo + nr, OUT:OUT + 2],
                                  in_=pcB[:nr])

            # Delay each tab2-chunk AllGather dispatch by a few windows so its
            # r2loc inputs are complete when it reaches the Pool queue head.
            CC2_DELAY = 4
            cc2_issue = {}
            for j in range(len(RB2) - 1):
                cc2_issue.setdefault(RB2[j] // PW - CC2_DELAY, []).append(j)

            def _emit_cc2(j):
                nc.gpsimd.collective_compute(
                    "AllGather", ALU.bypass,
                    replica_groups=[list(range(CORES))],
                    ins=[r2loc[RB2[j]:RB2[j + 1], :].opt()],
                    outs=[tab2[CORES * RB2[j]:CORES * RB2[j + 1], :].opt()])

            # ---------------- phase B: layer-1 edge phase -----------------
            for w in reversed(range(W)):
                KW = K[w]
                NI = KW * PW
                G1 = gp.tile([128, KW, REC1], BF16, tag="G1")
                nc.gpsimd.dma_gather(
                    G1[:], tab1[:, :], isrc_sb[:, CK[w] * 8:(CK[w] + KW) * 8],
                    NI, NI, REC1, single_packet=False)
                for j in cc2_issue.get(w, []):
                    _emit_cc2(j)
                G1v = G1[:, :, :].rearrange("p t (h c) -> p t h c", h=H)
                ysv = ysall[:, w, :].rearrange("p (h c) -> p h c", h=H)
                # ex = exp(norm * leaky_relu(y0[src] + ad'[dst]))  (pads -> ~0)
                es_t = mp.tile([128, KMAX, H], BF16, tag="es")
                es = es_t[:, :KW, :]
                nc.vector.tensor_tensor(
                    out=es.unsqueeze(3), in0=G1v[:, :, :, 0:1],
                    in1=adall[:, w, :].unsqueeze(1).unsqueeze(3)
                        .to_broadcast([128, KW, H, 1]),
                    op=ALU.add)
                nc.vector.scalar_tensor_tensor(
                    out=es, in0=es, scalar=NEG_SLOPE, in1=es,
                    op0=ALU.mult, op1=ALU.max)
                nc.vector.tensor_tensor(
                    out=es, in0=es,
                    in1=norms_sb[:].unsqueeze(1).to_broadcast([128, KW, H]),
                    op=ALU.mult)
                nc.scalar.activation(out=es, in_=es, func=ACTF.Exp)
                # self-loop term
                ess = mp.tile([128, H], F32, tag="ess")
                nc.vector.tensor_tensor(out=ess[:].unsqueeze(2),
                                        in0=ysv[:, :, 0:1],
                                        in1=adall[:, w, :].unsqueeze(2),
                                        op=ALU.add)
                nc.vector.scalar_tensor_tensor(
                    out=ess[:], in0=ess[:], scalar=NEG_SLOPE, in1=ess[:],
                    op0=ALU.mult, op1=ALU.max)
                nc.vector.tensor_tensor(out=ess[:], in0=ess[:],
                                        in1=norms_sb[:], op=ALU.mult)
                nc.scalar.activation(out=ess[:], in_=ess[:], func=ACTF.Exp)
                # denominator = sum_t ex + ex_self   (always >= ex_self > 0)
                den = mp.tile([128, H], F32, tag="den")
                nc.vector.tensor_reduce(
                    out=den[:], in_=es.rearrange("p t h -> p h t"),
                    axis=AXX, op=ALU.add)
                nc.vector.tensor_add(out=den[:], in0=den[:], in1=ess[:])
                rcp = mp.tile([128, H], F32, tag="rcp")
                nc.vector.reciprocal(rcp[:], den[:])
                # weighted feature sum over edge columns + self
                nc.vector.tensor_tensor(
                    out=G1v, in0=G1v,
                    in1=es.unsqueeze(3).to_broadcast([128, KW, H, C]),
                    op=ALU.mult)
                # halving-tree sum over the t axis (contiguous bf16 adds)
                cur = KW
                while cur > 1:
                    a = cur // 2
                    nc.vector.tensor_tensor(
                        out=G1[:, :a, :], in0=G1[:, :a, :],
                        in1=G1[:, cur - a:cur, :], op=ALU.add)
                    cur -= a
                tmp = mp.tile([128, F], F32, tag="tmp")
                nc.vector.tensor_tensor(
                    out=tmp[:].rearrange("p (h c) -> p h c", h=H),
                    in0=ysv,
                    in1=ess[:].unsqueeze(2).to_broadcast([128, H, C]),
                    op=ALU.mult)
                num = mp.tile([128, F], F32, tag="num")
                nc.vector.tensor_add(out=num[:], in0=G1[:, 0, :], in1=tmp[:])
                ho = mp.tile([128, F], F32, tag="ho")
                nc.vector.tensor_tensor(
                    out=ho[:].rearrange("p (h c) -> p h c", h=H),
                    in0=num[:].rearrange("p (h c) -> p h c", h=H),
                    in1=rcp[:].unsqueeze(2).to_broadcast([128, H, C]),
                    op=ALU.mult)
                # transpose y-aggregate, unrotate (h1 = y @ Q), ELU, and
                # produce rec2 = [h1 @ W2 | a2s | a2d]
                hT = mp.tile([128, KT, 128], F32, tag="hT")
                for g in range(KT):
                    tp = psA.tile([128, 128], F32, tag="tp")
                    nc.tensor.transpose(out=tp[:], in_=ho[:, g * 128:(g + 1) * 128],
                                        identity=ident[:])
                    nc.vector.tensor_copy(out=hT[:, g, :], in_=tp[:])
                h1e = mp.tile([128, KT, 128], F32, tag="h1e")
                xm = mp.tile([128, 128], F32, tag="xm")
                for jc in range(KT):
                    qp = psC.tile([128, 128], F32, tag="qp")
                    for g in range(KT):
                        nc.tensor.matmul(
                            qp[:], lhsT=qblk_sb[:, g, jc * 128:(jc + 1) * 128],
                            rhs=hT[:, g, :],
                            start=(g == 0), stop=(g == KT - 1))
                    # ELU(x) = relu(x) + exp(min(x,0)) - 1
                    nc.vector.tensor_scalar_min(out=xm[:], in0=qp[:], scalar1=0.0)
                    nc.scalar.activation(out=xm[:], in_=xm[:], func=ACTF.Exp)
                    nc.vector.tensor_scalar_max(out=h1e[:, jc, :], in0=qp[:],
                                                scalar1=0.0)
                    nc.vector.scalar_tensor_tensor(
                        out=h1e[:, jc, :], in0=h1e[:, jc, :], scalar=-1.0,
                        in1=xm[:], op0=ALU.add, op1=ALU.add)
                r2p = psB.tile([128, A2], F32, tag="acc2")
                for g in range(KT):
                    nc.tensor.matmul(r2p[:], lhsT=h1e[:, g, :], rhs=wc2_sb[:, g, :],
                                     start=(g == 0), stop=(g == KT - 1))
                r2sb = mp.tile([128, A2], F32, tag="r2sb")
                nc.vector.tensor_copy(out=r2sb[:], in_=r2p[:])
                NR = min(PW, NPC - w * PW)    # don't clobber the pad rows
                nc.sync.dma_start(out=r2loc[w * PW:w * PW + NR, :A2],
                                  in_=r2sb[:NR])

            for iw in sorted((i for i in cc2_issue if i < 0), reverse=True):
                for j in cc2_issue[iw]:
                    _emit_cc2(j)

            # ---------------- phase D: layer-2 edge phase -----------------
            # For the biggest windows, gather the column block whose sources
            # lie in tab2 rows >= TSPLIT (already AllGathered) before the
            # final tab2 chunks land; the remainder follows.
            split_tiles = {}
            for w in sorted(KA, reverse=True):
                kw2 = K2[w]
                G2 = gp2.tile([128, kw2, REC2], F32, tag="G2")
                split_tiles[w] = G2
                ka = KA[w]
                nc.gpsimd.dma_gather(
                    G2[:, :ka, :], tab2[TSPLIT:, :],
                    isrc2_sb[:, CK2[w] * 8:(CK2[w] + ka) * 8],
                    ka * PW, ka * PW, REC2, single_packet=False)
            for w in sorted(KA, reverse=True):
                G2 = split_tiles[w]
                ka = KA[w]
                kb = K2[w] - ka
                nc.gpsimd.dma_gather(
                    G2[:, ka:, :], tab2[:, :],
                    isrc2_sb[:, (CK2[w] + ka) * 8:(CK2[w] + ka + kb) * 8],
                    kb * PW, kb * PW, REC2, single_packet=False)
            for w in reversed(range(W)):
                KW = K2[w]
                NI = KW * PW
                if w in split_tiles:
                    G2 = split_tiles.pop(w)
                else:
                    G2 = gp2.tile([128, KW, REC2], F32, tag="G2")
                    nc.gpsimd.dma_gather(
                        G2[:], tab2[:, :],
                        isrc2_sb[:, CK2[w] * 8:(CK2[w] + KW) * 8],
                        NI, NI, REC2, single_packet=False)
                loc2 = mp.tile([128, A2], F32, tag="loc2")
                nc.sync.dma_start(out=loc2[:], in_=r2loc[w * PW:(w + 1) * PW, :A2])
                es2_t = mp.tile([128, KMAX, 1], F32, tag="es2")
                es2 = es2_t[:, :KW, :]
                nc.vector.tensor_tensor(
                    out=es2, in0=G2[:, :, OUT:OUT + 1],
                    in1=loc2[:, OUT + 1:OUT + 2].unsqueeze(1)
                        .to_broadcast([128, KW, 1]),
                    op=ALU.add)
                nc.vector.scalar_tensor_tensor(
                    out=es2, in0=es2, scalar=NEG_SLOPE, in1=es2,
                    op0=ALU.mult, op1=ALU.max)
                nc.scalar.activation(out=es2, in_=es2, func=ACTF.Exp)
                ess2 = mp.tile([128, 1], F32, tag="ess2")
                nc.vector.tensor_tensor(out=ess2[:], in0=loc2[:, OUT:OUT + 1],
                                        in1=loc2[:, OUT + 1:OUT + 2], op=ALU.add)
                nc.vector.scalar_tensor_tensor(
                    out=ess2[:], in0=ess2[:], scalar=NEG_SLOPE, in1=ess2[:],
                    op0=ALU.mult, op1=ALU.max)
                nc.scalar.activation(out=ess2[:], in_=ess2[:], func=ACTF.Exp)
                den2 = mp.tile([128, 1], F32, tag="den2")
                nc.vector.tensor_reduce(
                    out=den2[:], in_=es2.rearrange("p t h -> p h t"),
                    axis=AXX, op=ALU.add)
                nc.vector.tensor_add(out=den2[:], in0=den2[:], in1=ess2[:])
                rcp2 = mp.tile([128, 1], F32, tag="rcp2")
                nc.vector.reciprocal(rcp2[:], den2[:])
                nc.vector.tensor_tensor(
                    out=G2[:, :, :OUT], in0=G2[:, :, :OUT],
                    in1=es2.to_broadcast([128, KW, OUT]), op=ALU.mult)
                num2 = mp.tile([128, OUT], F32, tag="num2")
                nc.vector.tensor_reduce(
                    out=num2[:], in_=G2[:, :, :OUT].rearrange("p t f -> p f t"),
                    axis=AXX, op=ALU.add)
                tmp2 = mp.tile([128, OUT], F32, tag="tmp2")
                nc.vector.tensor_scalar_mul(out=tmp2[:], in0=loc2[:, :OUT],
                                            scalar1=ess2[:, :1])
                nc.vector.tensor_add(out=num2[:], in0=num2[:], in1=tmp2[:])
                nc.vector.tensor_scalar_mul(out=lgall[:, w, :], in0=num2[:],
                                            scalar1=rcp2[:, :1])
                if b2_p is not None:
                    nc.vector.tensor_add(out=lgall[:, w, :], in0=lgall[:, w, :],
                                         in1=b2_sb[:])

            # batched log_softmax over all windows (no max-shift: logits are
            # bounded); one ACT table load for the Exps, one Ln.
            for w in range(W):
                nc.scalar.activation(out=ezall[:, w, :], in_=lgall[:, w, :],
                                     func=ACTF.Exp, accum_out=ssall[:, w:w + 1])
            nc.scalar.activation(out=lsall[:], in_=ssall[:], func=ACTF.Ln)
            nc.vector.tensor_tensor(
                out=lgall[:], in0=lgall[:],
                in1=lsall[:].unsqueeze(2).to_broadcast([128, W, OUT]),
                op=ALU.subtract)
            nc.sync.dma_start(
                out=out_p[:W * PW].rearrange("(w p) o -> p w o", p=PW),
                in_=lgall[:])

    nc.compile()
    return nc


# --------------------------------------------------------------------------
# public entry point
# --------------------------------------------------------------------------

def kernel(x, edge_index, W1, a1_src, a1_dst, b1, W2, a2_src, a2_dst, b2,
           _want_trace=False):
    x = np.asarray(x)
    host, in_maps = _prep(x, np.asarray(edge_index), np.asarray(W1),
                          np.asarray(a1_src), np.asarray(a1_dst),
                          np.asarray(b1), np.asarray(W2), np.asarray(a2_src),
                          np.asarray(a2_dst), np.asarray(b2))
    key = (host["N"], host["F"], host["H"], host["C"], host["OUT"],
           host["K"], host["K2"], tuple(sorted(host["KA"].items())),
           host["use_b1"], host["use_b2"])
    if key not in _CACHE:
        _CACHE[key] = _build(host)
    nc = _CACHE[key]
    res = run_bass_kernel_spmd(nc, in_maps, core_ids=list(range(CORES)),
                               trace=_want_trace)
    NPC = host["NPC"]
    out = np.empty((host["N"], host["OUT"]), np.float32)
    for c in range(CORES):
        out[host["perm"][c * NPC:(c + 1) * NPC]] = res.results[c]["out"][:NPC]
    if _want_trace:
        kernel._last_result = res
    return np.ascontiguousarray(out)
